# revision 1
# baseline (speedup 1.0000x reference)
"""Trainium2 Bass kernel for per-view cross-attention.

Reference computation (per view v of 1024, S=64 samples, D=256):
  qp = q @ Wq.T + pe ; kp = k @ Wk.T + pe ; vp = v @ Wv.T + pe
  attn = softmax(qp @ kp.T / sqrt(D))
  x = gelu(attn @ vp @ Wo.T + bo) + q
Sharding: data-parallel over the 1024 views across 8 cores (128 views each).

On-chip layout strategy: everything is kept in "transposed" space [D, rows]
(rows = view*64+s) so that the contraction dim D lands on SBUF partitions
without any on-chip input transposes. The host pre-transposes q/k/v shards to
[D, rows] (free: numpy) and post-transposes the [D, rows] output back.
v additionally needs its projected form in natural [row, D] layout for the
attn@v matmul; that drops out naturally by using vT as the matmul stationary.
"""

import sys
import os

for p in ("/opt/trn_rl_repo",):
    if p not in sys.path and os.path.isdir(p):
        sys.path.insert(0, p)

import numpy as np

V, S, D = 1024, 64, 256
N_CORES = 8
VC = V // N_CORES          # views per core
ROWS = VC * S              # 8192 rows per core
R = 512                    # rows per supertile (8 views)
NST = ROWS // R            # supertiles per core
NV = R // S                # views per supertile
GELU_GROUP = 4             # supertiles per gelu flush (ACT table amortization)
PROJ_BUFS = 3
SM_BUFS = 3
PS_S_BUFS = 1
PS_T_BUFS = 1
PS_A_BUFS = 3
PS_B_BUFS = 3
LD_BUFS = 3
SCALE = 1.0 / np.sqrt(np.float32(D)).astype(np.float32)

_CACHE = {}


def _make_posenc(d_hid, n_samples):
    pos = np.arange(n_samples, dtype=np.float64)[:, None]
    j = np.arange(d_hid)[None, :]
    angle = pos / np.power(10000.0, 2.0 * (j // 2) / d_hid)
    table = np.where(j % 2 == 0, np.sin(angle), np.cos(angle))
    return table.astype(np.float32)  # [S, D]


def _build(rows=ROWS, stage=99):
    import concourse.bass as bass
    import concourse.mybir as mybir
    import concourse.tile as tile
    from concourse.tile import add_dep_helper
    from concourse import bacc
    from contextlib import ExitStack

    fp32 = mybir.dt.float32
    f32r = mybir.dt.float32r
    bf16 = mybir.dt.bfloat16
    AF = mybir.ActivationFunctionType
    ALU = mybir.AluOpType
    n_st = rows // R

    nc = bacc.Bacc(None, target_bir_lowering=False)

    qT_d = nc.dram_tensor("qT", [D, rows], f32r, kind="ExternalInput")
    kT_d = nc.dram_tensor("kT", [D, rows], f32r, kind="ExternalInput")
    vT_d = nc.dram_tensor("vT", [D, rows], f32r, kind="ExternalInput")
    wq_d = nc.dram_tensor("WqT", [D, D], f32r, kind="ExternalInput")
    wk_d = nc.dram_tensor("WkT", [D, D], f32r, kind="ExternalInput")
    wv_d = nc.dram_tensor("WvT", [D, D], f32r, kind="ExternalInput")
    wo_d = nc.dram_tensor("WoT", [D, D], f32r, kind="ExternalInput")
    bo_d = nc.dram_tensor("bo", [D], fp32, kind="ExternalInput")
    pet_d = nc.dram_tensor("peT_rep", [D, R], fp32, kind="ExternalInput")
    pe_d = nc.dram_tensor("pe_nat", [S, D], f32r, kind="ExternalInput")
    e2_d = nc.dram_tensor("E2", [S, 128], f32r, kind="ExternalInput")
    id_d = nc.dram_tensor("I128", [128, 128], fp32, kind="ExternalInput")
    out_d = nc.dram_tensor("outT", [D, rows], fp32, kind="ExternalOutput")

    def r3(ap):  # [D, X] dram -> [128, 2, X] partition view
        return ap.rearrange("(kc p) r -> p kc r", p=128)

    with tile.TileContext(nc) as tc, ExitStack() as ctx:
        const = ctx.enter_context(tc.tile_pool(name="const", bufs=1))
        ld = ctx.enter_context(tc.tile_pool(name="ld", bufs=LD_BUFS))
        proj = ctx.enter_context(tc.tile_pool(name="proj", bufs=PROJ_BUFS))
        sm = ctx.enter_context(tc.tile_pool(name="sm", bufs=SM_BUFS))
        psA = ctx.enter_context(tc.tile_pool(name="psA", bufs=PS_A_BUFS, space="PSUM"))
        psB = ctx.enter_context(tc.tile_pool(name="psB", bufs=PS_B_BUFS, space="PSUM"))
        psS = ctx.enter_context(tc.tile_pool(name="psS", bufs=PS_S_BUFS, space="PSUM"))
        psT = ctx.enter_context(tc.tile_pool(name="psT", bufs=PS_T_BUFS, space="PSUM"))
        stg = ctx.enter_context(tc.tile_pool(name="stg", bufs=GELU_GROUP + 1))

        wq = const.tile([128, 2, D], f32r)
        wk = const.tile([128, 2, D], f32r)
        wv = const.tile([128, 2, D], f32r)
        wo = const.tile([128, 2, D], f32r)
        nc.sync.dma_start(wq, r3(wq_d[:]))
        nc.sync.dma_start(wk, r3(wk_d[:]))
        nc.sync.dma_start(wv, r3(wv_d[:]))
        nc.sync.dma_start(wo, r3(wo_d[:]))
        pet = const.tile([128, 2, R], fp32)
        nc.sync.dma_start(pet, r3(pet_d[:]))
        pe_sb = const.tile([S, D], f32r)
        nc.sync.dma_start(pe_sb, pe_d[:])
        e2 = const.tile([S, 128], f32r)
        nc.sync.dma_start(e2, e2_d[:])
        i128 = const.tile([128, 128], fp32)
        nc.sync.dma_start(i128, id_d[:])
        bo_sb = const.tile([128, 2], fp32)
        nc.sync.dma_start(bo_sb, bo_d.rearrange("(kc p) -> p kc", p=128))

        pending = []
        last_gelu = None
        last_exp = None
        for st in range(n_st):
            rs = slice(st * R, (st + 1) * R)
            qt = ld.tile([128, 2, R], f32r, tag="qt", bufs=GELU_GROUP + 2)
            kt = ld.tile([128, 2, R], f32r, tag="kt")
            vt = ld.tile([128, 2, R], f32r, tag="vt")
            nc.sync.dma_start(qt, r3(qT_d[:])[:, :, rs])
            nc.sync.dma_start(kt, r3(kT_d[:])[:, :, rs])
            nc.sync.dma_start(vt, r3(vT_d[:])[:, :, rs])

            # ---- projections into transposed space: xpT[dout, row] ----
            qpT = proj.tile([128, 2, R], fp32, tag="qpT")
            kpT = proj.tile([128, 2, R], fp32, tag="kpT")
            for w_sb, x_sb, o_sb in ((wq, qt, qpT), (wk, kt, kpT)):
                for mc in range(2):
                    ps = psA.tile([128, R], fp32, tag="psA", name="ps_proj")
                    for kc in range(2):
                        nc.tensor.matmul(
                            ps,
                            w_sb[:, kc, mc * 128:(mc + 1) * 128],
                            x_sb[:, kc, :],
                            start=(kc == 0),
                            stop=(kc == 1),
                        )
                    # evacuate PSUM fused with positional-encoding add
                    nc.vector.tensor_add(
                        out=o_sb[:, mc, :], in0=ps, in1=pet[:, mc, :]
                    )

            if stage <= 1:
                nc.sync.dma_start(r3(out_d[:])[:, :, rs], qpT)
                continue
            # ---- vp in natural [row, dout] layout (vT as stationary) ----
            vp = proj.tile([128, 4, D], fp32, tag="vp")
            for g in range(4):
                psv = psB.tile([128, D], fp32, tag="psB", name="ps_vp")
                for kc in range(2):
                    nc.tensor.matmul(
                        psv,
                        vt[:, kc, g * 128:(g + 1) * 128],
                        wv[:, kc, :],
                        start=(kc == 0),
                        stop=False,
                    )
                # pe add folded in as a matmul: E2.T @ pe = pe tiled over rows
                nc.tensor.matmul(psv, e2, pe_sb, start=False, stop=True)
                nc.scalar.copy(out=vp[:, g, :], in_=psv)

            if stage <= 2:
                nc.sync.dma_start(r3(out_d[:])[:, :, rs], vp.rearrange("p a b -> p (a b)")[:, None, :].rearrange("p o (a b) -> p (o a) b", a=2))
                continue
            # ---- scores: per view [64,64], packed [128(2 views), 4, 64] ----
            scps = psS.tile([128, 4, S], fp32, tag="scores")
            for v in range(NV):
                g, h = v // 2, v % 2
                for dc in range(2):
                    nc.tensor.matmul(
                        scps[h * 64:(h + 1) * 64, g, :],
                        qpT[:, dc, v * S:(v + 1) * S],
                        kpT[:, dc, v * S:(v + 1) * S],
                        start=(dc == 0),
                        stop=(dc == 1),
                        tile_position=(0, h * 64),
                    )

            # ---- softmax along free axis (no max-subtraction: |scores/16|<~10) ----
            attn = sm.tile([128, 4, S], fp32, tag="attn")
            _e = nc.scalar.activation(attn, scps, AF.Exp, scale=float(SCALE))
            # keep Exp-set ops contiguous on ACT: exp of a new gelu-group must
            # come after the previous group's last gelu
            if last_gelu is not None:
                add_dep_helper(_e.ins, last_gelu, sync=False,
                               reason="act-table grouping: exp after prior gelus")
            last_exp = _e.ins
            sums = sm.tile([128, 4], fp32, tag="sums")
            nc.vector.tensor_reduce(out=sums, in_=attn, axis=mybir.AxisListType.X, op=ALU.add)
            rec = sm.tile([128, 4], fp32, tag="rec")
            nc.vector.reciprocal(rec, sums)
            nc.vector.tensor_tensor(
                attn, attn, rec[:, :, None].to_broadcast((128, 4, S)), ALU.mult
            )

            if stage <= 3:
                nc.sync.dma_start(r3(out_d[:])[:, 0, st * R: st * R + 256], attn.rearrange("p a b -> p (a b)"))
                continue
            # ---- transpose attn packs; duplicate into both partition halves ----
            atps = psT.tile([128, 4, 128], fp32, tag="attnT")
            for g in range(4):
                for h in range(2):
                    nc.tensor.matmul(
                        atps[h * 64:(h + 1) * 64, g, :],
                        attn[:, g, :],
                        i128,
                        start=True,
                        stop=True,
                        tile_position=(0, h * 64),
                    )
            attnT = sm.tile([128, 4, 128], fp32, tag="attnT_sb")
            nc.scalar.copy(out=attnT, in_=atps)

            if stage <= 4:
                nc.sync.dma_start(r3(out_d[:])[:, 0, st * R: st * R + 512], attnT.rearrange("p a b -> p (a b)"))
                continue
            # ---- attn @ vp, directly in transposed space outT[d, row] ----
            # Concurrent row-group matmuls must not drain into the same
            # (partition, bank) pair: one PSUM tile per row-half h.
            outT = proj.tile([128, 2, R], f32r, tag="outT")
            for c in range(2):
                for h in range(2):
                    pso = psB.tile([128, 4, S], fp32, tag="psB", name="ps_av")
                    for g in range(4):
                        nc.tensor.matmul(
                            pso[:, g, :],
                            vp[h * 64:(h + 1) * 64, g, c * 128:(c + 1) * 128],
                            attnT[h * 64:(h + 1) * 64, g, h * 64:(h + 1) * 64],
                            start=True,
                            stop=True,
                            tile_position=(h * 64, 0),
                        )
                    # view v=2g+h lives at free offset v*64 of outT chunk c
                    o_ap = outT[:, c, :].rearrange(
                        "p (g two s) -> p g two s", two=2, s=S
                    )[:, :, h, :]
                    if c == 0:
                        nc.vector.tensor_copy(o_ap, pso)
                    else:
                        nc.scalar.copy(out=o_ap, in_=pso)

            if stage <= 5:
                nc.sync.dma_start(r3(out_d[:])[:, :, rs], outT)
                continue
            # ---- final projection, staged pre-gelu (Exp and Gelu live in
            # different ACT table sets; group gelus to amortize ~2.7us
            # table switches) ----
            pre = stg.tile([128, 2, R], fp32, tag="pre")
            for mc in range(2):
                psf = psA.tile([128, R], fp32, tag="psA", name="ps_fin")
                for kc in range(2):
                    nc.tensor.matmul(
                        psf,
                        wo[:, kc, mc * 128:(mc + 1) * 128],
                        outT[:, kc, :],
                        start=(kc == 0),
                        stop=(kc == 1),
                    )
                if mc == 0:
                    nc.vector.tensor_copy(pre[:, mc, :], psf)
                else:
                    nc.scalar.copy(out=pre[:, mc, :], in_=psf)
            pending.append((st, pre, qt))

            if len(pending) == GELU_GROUP or st == n_st - 1:
                for pst, ppre, pqt in pending:
                    outsb = proj.tile([128, 2, R], fp32, tag="outsb")
                    for mc in range(2):
                        _g = nc.scalar.activation(
                            out=outsb[:, mc, :], in_=ppre[:, mc, :],
                            func=AF.Gelu, bias=bo_sb[:, mc:mc + 1], scale=1.0,
                        )
                        if last_exp is not None:
                            add_dep_helper(_g.ins, last_exp, sync=False,
                                           reason="act-table grouping: gelu after group exps")
                        last_gelu = _g.ins
                        nc.vector.tensor_add(
                            out=outsb[:, mc, :], in0=outsb[:, mc, :],
                            in1=pqt[:, mc, :],
                        )
                    nc.sync.dma_start(
                        r3(out_d[:])[:, :, pst * R:(pst + 1) * R], outsb
                    )
                pending = []

    nc.finalize()
    return nc


def _get_nc():
    if "nc" not in _CACHE:
        _CACHE["nc"] = _build()
    return _CACHE["nc"]


def _host_inputs(q, k, v, Wq, Wk, Wv, Wo, bo):
    pe = _make_posenc(D, S)                      # [S, D]
    peT_rep = np.ascontiguousarray(np.tile(pe.T, (1, NV)))   # [D, R]
    e2 = np.ascontiguousarray(np.tile(np.eye(S, dtype=np.float32), (1, 2)))
    i128 = np.eye(128, dtype=np.float32)
    consts = {
        "WqT": np.ascontiguousarray(np.asarray(Wq, np.float32).T),
        "WkT": np.ascontiguousarray(np.asarray(Wk, np.float32).T),
        "WvT": np.ascontiguousarray(np.asarray(Wv, np.float32).T),
        "WoT": np.ascontiguousarray(np.asarray(Wo, np.float32).T),
        "bo": np.ascontiguousarray(np.asarray(bo, np.float32)),
        "peT_rep": peT_rep,
        "pe_nat": pe,
        "E2": e2,
        "I128": i128,
    }
    in_maps = []
    for c in range(N_CORES):
        sl = slice(c * VC, (c + 1) * VC)
        m = dict(consts)
        m["qT"] = np.ascontiguousarray(
            np.asarray(q, np.float32)[sl].reshape(ROWS, D).T)
        m["kT"] = np.ascontiguousarray(
            np.asarray(k, np.float32)[sl].reshape(ROWS, D).T)
        m["vT"] = np.ascontiguousarray(
            np.asarray(v, np.float32)[sl].reshape(ROWS, D).T)
        in_maps.append(m)
    return in_maps


def kernel(q, k, v, Wq, Wk, Wv, Wo, bo, _trace=False):
    from concourse.bass_utils import run_bass_kernel_spmd

    nc = _get_nc()
    in_maps = _host_inputs(q, k, v, Wq, Wk, Wv, Wo, bo)
    res = run_bass_kernel_spmd(nc, in_maps, list(range(N_CORES)), trace=_trace)
    outs = [
        res.results[c]["outT"].reshape(D, VC, S).transpose(1, 2, 0)
        for c in range(N_CORES)
    ]
    full = np.concatenate(outs, axis=0)
    if _trace:
        _CACHE["last_results"] = res
    return full



# revision 6
# speedup vs baseline: 1.7244x; 1.7244x over previous
"""Trainium2 Bass kernel for per-view cross-attention (v3, bf16 + fused Wo).

Reference computation (per view v of 1024, S=64 samples, D=256):
  qp = q @ Wq.T + pe ; kp = k @ Wk.T + pe ; vp = v @ Wv.T + pe
  attn = softmax(qp @ kp.T / sqrt(D))
  x = gelu(attn @ vp @ Wo.T + bo) + q
Sharding: data-parallel over the 1024 views across 8 cores (128 views each).

Design notes:
- bf16 everywhere (DRAM I/O, SBUF, matmul operands; PSUM stays fp32).
  Halves HBM traffic and dodges the 4x cycles/row penalty on fp32 matmuls
  with small output free dims (scores, attn@v).
- Wo is folded into the v path on the host: vpo = v@(Wo@Wv).T + pe@Wo.T,
  and out = attn@vpo directly gives the pre-gelu activation (attn row-mixing
  commutes with the Wo column-mixing). Kills the final projection matmuls
  and one full PSUM evacuation round.
- scores are computed TRANSPOSED (operands swapped), so no attn transpose
  is needed before attn@v. Softmax reduction runs along partitions on PE:
  denominators via a [128,2] ones matmul, reciprocal on DVE, broadcast back
  across partitions with a [2,128] ones outer-product matmul.
- 4-stage software pipeline (A: load/proj/scoresT/exp; B: sums+recip;
  C: rec-broadcast+normalize; D: attn@vpo + gelu flush) so the in-order PE
  never waits on the ACT/DVE softmax chain.
- engine balance per supertile (cost model): PE ~4.0us, DVE ~3.8, ACT ~3.5,
  Pool ~2.0. Pool (gpsimd) cannot access PSUM, so it gets the SBUF-only
  residual adds; pos-enc adds ride PSUM evacuations on DVE, except kp's,
  which is folded into its projection as a third accumulation matmul.
"""

import sys
import os

for p in ("/opt/trn_rl_repo",):
    if p not in sys.path and os.path.isdir(p):
        sys.path.insert(0, p)

import numpy as np
import ml_dtypes

BF16 = ml_dtypes.bfloat16

V, S, D = 1024, 64, 256
N_CORES = 8
VC = V // N_CORES          # views per core
ROWS = VC * S              # 8192 rows per core
R = 512                    # rows per supertile (8 views)
NST = ROWS // R            # supertiles per core
NV = R // S                # views per supertile
GELU_GROUP = 4             # supertiles per gelu flush (ACT table amortization)
LAG_B, LAG_C, LAG_D = 1, 2, 3
SCALE = 1.0 / np.sqrt(np.float32(D)).astype(np.float32)

_CACHE = {}


def _make_posenc(d_hid, n_samples):
    pos = np.arange(n_samples, dtype=np.float64)[:, None]
    j = np.arange(d_hid)[None, :]
    angle = pos / np.power(10000.0, 2.0 * (j // 2) / d_hid)
    table = np.where(j % 2 == 0, np.sin(angle), np.cos(angle))
    return table.astype(np.float32)  # [S, D]


def _build(rows=ROWS):
    import concourse.bass as bass
    import concourse.mybir as mybir
    import concourse.tile as tile
    from concourse.tile import add_dep_helper
    from concourse import bacc
    from contextlib import ExitStack

    fp32 = mybir.dt.float32
    bf16 = mybir.dt.bfloat16
    AF = mybir.ActivationFunctionType
    ALU = mybir.AluOpType
    n_st = rows // R

    nc = bacc.Bacc(None, target_bir_lowering=False)

    qT_d = nc.dram_tensor("qT", [D, rows], bf16, kind="ExternalInput")
    kT_d = nc.dram_tensor("kT", [D, rows], bf16, kind="ExternalInput")
    vT_d = nc.dram_tensor("vT", [D, rows], bf16, kind="ExternalInput")
    wq_d = nc.dram_tensor("WqT", [D, D], bf16, kind="ExternalInput")
    wk_d = nc.dram_tensor("WkT", [D, D], bf16, kind="ExternalInput")
    wvo_d = nc.dram_tensor("WvoT", [D, D], bf16, kind="ExternalInput")
    bo_d = nc.dram_tensor("bo", [D], fp32, kind="ExternalInput")
    pet_d = nc.dram_tensor("peT_rep", [D, R], bf16, kind="ExternalInput")
    pen_d = nc.dram_tensor("pe_nat", [S, D], bf16, kind="ExternalInput")
    peo2_d = nc.dram_tensor("peo_nat2", [128, D], bf16, kind="ExternalInput")
    e8_d = nc.dram_tensor("E8", [S, R], bf16, kind="ExternalInput")
    ones2_d = nc.dram_tensor("ones2", [128, 2], bf16, kind="ExternalInput")
    ones2t_d = nc.dram_tensor("ones2T", [2, 128], bf16, kind="ExternalInput")
    out_d = nc.dram_tensor("outT", [D, rows], bf16, kind="ExternalOutput")

    def r3(ap):  # [D, X] dram -> [128, 2, X] partition view
        return ap.rearrange("(kc p) r -> p kc r", p=128)

    with tile.TileContext(nc) as tc, ExitStack() as ctx:
        ctx.enter_context(nc.allow_low_precision(
            reason="bf16 throughout is within the 2e-2 rel-err budget"))
        const = ctx.enter_context(tc.tile_pool(name="const", bufs=1))
        ld = ctx.enter_context(tc.tile_pool(name="ld", bufs=3))
        proj = ctx.enter_context(tc.tile_pool(name="proj", bufs=2))
        sm = ctx.enter_context(tc.tile_pool(name="sm", bufs=3))
        stg = ctx.enter_context(tc.tile_pool(name="stg", bufs=GELU_GROUP + 1))
        psA = ctx.enter_context(tc.tile_pool(name="psA", bufs=2, space="PSUM"))
        psB = ctx.enter_context(tc.tile_pool(name="psB", bufs=3, space="PSUM"))
        psS = ctx.enter_context(tc.tile_pool(name="psS", bufs=1, space="PSUM"))
        psR = ctx.enter_context(tc.tile_pool(name="psR", bufs=1, space="PSUM"))

        wq = const.tile([128, 2, D], bf16)
        wk = const.tile([128, 2, D], bf16)
        wvo = const.tile([128, 2, D], bf16)
        nc.sync.dma_start(wq, r3(wq_d[:]))
        nc.sync.dma_start(wk, r3(wk_d[:]))
        nc.sync.dma_start(wvo, r3(wvo_d[:]))
        pet = const.tile([128, 2, R], bf16)
        nc.sync.dma_start(pet, r3(pet_d[:]))
        pen = const.tile([S, D], bf16)
        nc.sync.dma_start(pen, pen_d[:])
        peo2 = const.tile([128, D], bf16)
        nc.sync.dma_start(peo2, peo2_d[:])
        e8 = const.tile([S, R], bf16)
        nc.sync.dma_start(e8, e8_d[:])
        ones2 = const.tile([128, 2], bf16)
        nc.sync.dma_start(ones2, ones2_d[:])
        ones2t = const.tile([2, 128], bf16)
        nc.sync.dma_start(ones2t, ones2t_d[:])
        bo_sb = const.tile([128, 2], fp32)
        nc.sync.dma_start(bo_sb, bo_d.rearrange("(kc p) -> p kc", p=128))

        st_ctx = {}
        pending = []
        last_gelu = None
        last_exp = None
        for i in range(n_st + LAG_D):
            # -------- stage A: load, q/k/v projections, scoresT, exp --------
            if i < n_st:
                rs = slice(i * R, (i + 1) * R)
                qt = ld.tile([128, 2, R], bf16, tag="qt",
                             bufs=LAG_D + GELU_GROUP + 2, name="qt")
                kt = ld.tile([128, 2, R], bf16, tag="kt", name="kt")
                vt = ld.tile([128, 2, R], bf16, tag="vt", name="vt")
                nc.sync.dma_start(qt, r3(qT_d[:])[:, :, rs])
                nc.sync.dma_start(kt, r3(kT_d[:])[:, :, rs])
                nc.sync.dma_start(vt, r3(vT_d[:])[:, :, rs])

                # projections into transposed space xpT[dout, row].
                # qp: pos-enc add rides the DVE evacuation.
                # kp: pos-enc added on PE as a 3rd accumulation matmul
                #     (pe_nat as stationary, E8 one-hot rhs); ACT evacuates.
                qpT = proj.tile([128, 2, R], bf16, tag="qpT", name="qpT")
                kpT = proj.tile([128, 2, R], bf16, tag="kpT", name="kpT")
                for mc in range(2):
                    ps = psA.tile([128, R], fp32, tag="psA", name="ps_qp")
                    for kc in range(2):
                        nc.tensor.matmul(
                            ps,
                            wq[:, kc, mc * 128:(mc + 1) * 128],
                            qt[:, kc, :],
                            start=(kc == 0),
                            stop=(kc == 1),
                        )
                    nc.vector.tensor_add(
                        out=qpT[:, mc, :], in0=ps, in1=pet[:, mc, :])
                for mc in range(2):
                    ps = psA.tile([128, R], fp32, tag="psA", name="ps_kp")
                    for kc in range(2):
                        nc.tensor.matmul(
                            ps,
                            wk[:, kc, mc * 128:(mc + 1) * 128],
                            kt[:, kc, :],
                            start=(kc == 0),
                            stop=False,
                        )
                    nc.tensor.matmul(
                        ps,
                        pen[:, mc * 128:(mc + 1) * 128],
                        e8,
                        start=False,
                        stop=True,
                    )
                    nc.scalar.copy(out=kpT[:, mc, :], in_=ps)

                # vpo = v@(Wo@Wv).T + pe@Wo.T, natural [row, dout] layout
                # (vt chunk as stationary); pos-enc add rides the evacuation.
                vpo = proj.tile([128, 4, D], bf16, tag="vpo",
                                bufs=LAG_D + 2, name="vpo")
                for g in range(4):
                    psv = psB.tile([128, D], fp32, tag="psB", name="ps_vpo")
                    for kc in range(2):
                        nc.tensor.matmul(
                            psv,
                            vt[:, kc, g * 128:(g + 1) * 128],
                            wvo[:, kc, :],
                            start=(kc == 0),
                            stop=(kc == 1),
                        )
                    nc.vector.tensor_add(out=vpo[:, g, :], in0=psv, in1=peo2)

                # transposed scores: scps[k, q] per view, packed
                # [128(2 views k), 4, 64]
                scps = psS.tile([128, 4, S], fp32, tag="scps", name="scps")
                for v in range(NV):
                    g, h = v // 2, v % 2
                    for dc in range(2):
                        nc.tensor.matmul(
                            scps[h * 64:(h + 1) * 64, g, :],
                            kpT[:, dc, v * S:(v + 1) * S],
                            qpT[:, dc, v * S:(v + 1) * S],
                            start=(dc == 0),
                            stop=(dc == 1),
                            tile_position=(0, h * 64),
                        )

                # exp (no max-subtraction: |scores/16| < ~10)
                attnu = sm.tile([128, 4, S], bf16, tag="attnu",
                                bufs=LAG_C + 2, name="attnu")
                _e = nc.scalar.activation(attnu, scps, AF.Exp, scale=float(SCALE))
                if last_gelu is not None:
                    add_dep_helper(_e.ins, last_gelu, sync=False,
                                   reason="act-table grouping: exp after prior gelus")
                last_exp = _e.ins
                st_ctx[i] = dict(qt=qt, vpo=vpo, attnu=attnu)

            # -------- stage B: softmax denominators (PE) + reciprocal ------
            jb = i - LAG_B
            if 0 <= jb < n_st:
                c = st_ctx[jb]
                sums = psR.tile([2, 4, S], fp32, tag="sums", name="sums")
                for g in range(4):
                    nc.tensor.matmul(
                        sums[:, g, :], ones2, c["attnu"][:, g, :],
                        start=True, stop=True,
                    )
                rec = sm.tile([2, 4, S], bf16, tag="rec", name="rec")
                nc.vector.reciprocal(rec, sums)
                c["rec"] = rec

            # -------- stage C: broadcast reciprocal, normalize -------------
            jc = i - LAG_C
            if 0 <= jc < n_st:
                c = st_ctx[jc]
                rrep = psR.tile([128, 4, S], fp32, tag="rrep", name="rrep")
                for g in range(4):
                    nc.tensor.matmul(
                        rrep[:, g, :], ones2t, c["rec"][:, g, :],
                        start=True, stop=True,
                    )
                attnT = sm.tile([128, 4, S], bf16, tag="attnT", name="attnT")
                nc.vector.tensor_tensor(attnT, c["attnu"], rrep, ALU.mult)
                c["attnT"] = attnT

            # -------- stage D: attn@vpo -> pre-gelu, gelu flush ------------
            jd = i - LAG_D
            if 0 <= jd < n_st:
                c = st_ctx.pop(jd)
                pre = stg.tile([128, 2, R], bf16, tag="pre", name="pre")
                for cc in range(2):
                    for h in range(2):
                        pso = psB.tile([128, 4, S], fp32, tag="psB", name="ps_av")
                        for g in range(4):
                            nc.tensor.matmul(
                                pso[:, g, :],
                                c["vpo"][h * 64:(h + 1) * 64, g, cc * 128:(cc + 1) * 128],
                                c["attnT"][h * 64:(h + 1) * 64, g, :],
                                start=True,
                                stop=True,
                                tile_position=(h * 64, 0),
                            )
                        # view v=2g+h lives at free offset v*64 of chunk cc
                        o_ap = pre[:, cc, :].rearrange(
                            "p (g two s) -> p g two s", two=2, s=S
                        )[:, :, h, :]
                        if cc == 0:
                            nc.scalar.copy(out=o_ap, in_=pso)
                        else:
                            nc.vector.tensor_copy(o_ap, pso)
                pending.append((jd, pre, c["qt"]))

                if len(pending) == GELU_GROUP or jd == n_st - 1:
                    outs = []
                    for pst, ppre, pqt in pending:
                        outsb = proj.tile([128, 2, R], bf16, tag="outsb",
                                          bufs=GELU_GROUP + 1, name="outsb")
                        for mc in range(2):
                            _g = nc.scalar.activation(
                                out=outsb[:, mc, :], in_=ppre[:, mc, :],
                                func=AF.Gelu, bias=bo_sb[:, mc:mc + 1], scale=1.0,
                            )
                            if last_exp is not None:
                                add_dep_helper(_g.ins, last_exp, sync=False,
                                               reason="act-table grouping: gelu after group exps")
                            last_gelu = _g.ins
                            nc.gpsimd.tensor_add(
                                out=outsb[:, mc, :], in0=outsb[:, mc, :],
                                in1=pqt[:, mc, :],
                            )
                        outs.append((pst, outsb))
                    for pst, outsb in outs:
                        nc.scalar.dma_start(
                            r3(out_d[:])[:, :, pst * R:(pst + 1) * R], outsb
                        )
                    pending = []

    nc.finalize()
    return nc


def _get_nc():
    if "nc" not in _CACHE:
        _CACHE["nc"] = _build()
    return _CACHE["nc"]


def _host_inputs(q, k, v, Wq, Wk, Wv, Wo, bo):
    pe = _make_posenc(D, S)                                   # [S, D] fp32
    Wo32 = np.asarray(Wo, np.float32)
    Wv32 = np.asarray(Wv, np.float32)
    Wvo = Wo32 @ Wv32                                         # fused v->out
    peo = pe @ Wo32.T                                         # pe through Wo
    peT_rep = np.ascontiguousarray(np.tile(pe.T, (1, NV))).astype(BF16)
    peo2 = np.ascontiguousarray(np.tile(peo, (2, 1))).astype(BF16)  # [128, D]
    e8 = np.ascontiguousarray(
        np.tile(np.eye(S, dtype=np.float32), (1, NV))).astype(BF16)  # [S, R]
    ones2 = np.zeros((128, 2), BF16)
    ones2[:64, 0] = 1
    ones2[64:, 1] = 1
    ones2t = np.ascontiguousarray(ones2.T)                    # [2, 128]
    consts = {
        "WqT": np.asarray(Wq, np.float32).T.astype(BF16),
        "WkT": np.asarray(Wk, np.float32).T.astype(BF16),
        "WvoT": Wvo.T.astype(BF16),
        "bo": np.ascontiguousarray(np.asarray(bo, np.float32)),
        "peT_rep": peT_rep,
        "pe_nat": pe.astype(BF16),
        "peo_nat2": peo2,
        "E8": e8,
        "ones2": ones2,
        "ones2T": ones2t,
    }
    consts = {k_: np.ascontiguousarray(v_) for k_, v_ in consts.items()}
    qb = np.asarray(q, np.float32).astype(BF16)
    kb = np.asarray(k, np.float32).astype(BF16)
    vb = np.asarray(v, np.float32).astype(BF16)
    in_maps = []
    for c in range(N_CORES):
        sl = slice(c * VC, (c + 1) * VC)
        m = dict(consts)
        m["qT"] = np.ascontiguousarray(qb[sl].reshape(ROWS, D).T)
        m["kT"] = np.ascontiguousarray(kb[sl].reshape(ROWS, D).T)
        m["vT"] = np.ascontiguousarray(vb[sl].reshape(ROWS, D).T)
        in_maps.append(m)
    return in_maps


def kernel(q, k, v, Wq, Wk, Wv, Wo, bo, _trace=False):
    from concourse.bass_utils import run_bass_kernel_spmd

    nc = _get_nc()
    in_maps = _host_inputs(q, k, v, Wq, Wk, Wv, Wo, bo)
    res = run_bass_kernel_spmd(nc, in_maps, list(range(N_CORES)), trace=_trace)
    outs = [
        np.asarray(res.results[c]["outT"], np.float32)
        .reshape(D, VC, S).transpose(1, 2, 0)
        for c in range(N_CORES)
    ]
    full = np.concatenate(outs, axis=0)
    if _trace:
        _CACHE["last_results"] = res
    return full
